# revision 10
# baseline (speedup 1.0000x reference)
"""Label-wise FFN kernel for Trainium2 (8 NeuronCores, label-sharded).

Computes out[b, l] = relu(x @ W1[l] + b1[l]) @ W2[l] + b2[l] for
B=8192, D=1024, L=64, H=256, fp32 in/out. rel-tol 2e-2 allows bf16
matmul operands (measured ~3e-3 end-to-end).

Sharding: L is split across the 8 cores (8 labels each); every core holds a
full replica of x. Per-core output is [B, 8]; host concatenates along L.

Per-core dataflow (variant "b", x-stationary):
  layer 1: for each b-tile (128 rows), stationary xT-tile [d=128, b=128]
           streams W1 moving tiles [d=128, 512] (2 labels x 256h packed on
           the free dim) -> psum[b=128, 512] accumulated over 8 d-tiles.
           bf16 operands enable fast-weight-load so the PE stays at the
           ~213ns/512-col streaming rate (f32r paid ~280ns fused loads).
  epilog:  relu(p + b1) = max(p, -b1) + b1, and sum_h w2*b1 is folded into
           b2 on the host. So DVE does: hmax = max(psum, -b1) (PSUM->SBUF,
           bf16), then per label a fused tensor_tensor_reduce
           out[b,l] = sum_h hmax*w2 + b2'. Layer 2 never touches the PE,
           which only does the 2048 layer-1 matmuls (~437us streaming
           floor).

Variant "a" is the previous schedule (W-stationary, h on partitions,
layer 2 as block-diagonal PE matmuls) with operands in bf16.
"""

import numpy as np
import ml_dtypes

import concourse.bacc as bacc
import concourse.mybir as mybir
import concourse.tile as tile
from concourse.bass_utils import run_bass_kernel_spmd

B, D, L, H = 8192, 1024, 64, 256
NCORES = 8
LPC = L // NCORES      # labels per core
P = 128
KT = D // P            # k-tiles over D
PAIRS = LPC // 2       # 2 labels (2x256h) packed per 512-wide moving tile
NBT = B // P           # b-tiles (variant b)
BCHUNK = 512
NB = B // BCHUNK       # b-chunks (variant a)
HC = H // P            # h-chunks per label (variant a)
NL2 = LPC * HC

BF16 = mybir.dt.bfloat16
F32 = mybir.dt.float32
NP_BF16 = ml_dtypes.bfloat16


def build_nc(variant="b", repeat=1, x_bufs=3, ps_bufs=8, h_bufs=6, s_bufs=3,
             o_bufs=3):
    if variant == "a":
        return _build_a(repeat)
    nc = bacc.Bacc(None, target_bir_lowering=False)

    xT = nc.dram_tensor("xT", [D, B], BF16, kind="ExternalInput")
    w1t = nc.dram_tensor("w1t", [PAIRS, KT, P, 2 * H], BF16, kind="ExternalInput")
    nb1 = nc.dram_tensor("nb1", [P, PAIRS, 2 * H], BF16, kind="ExternalInput")
    w2r = nc.dram_tensor("w2r", [P, PAIRS, 2 * H], BF16, kind="ExternalInput")
    b2p = nc.dram_tensor("b2p", [P, LPC], F32, kind="ExternalInput")
    out = nc.dram_tensor("out", [B, LPC], F32, kind="ExternalOutput")

    mult = mybir.AluOpType.mult
    add = mybir.AluOpType.add
    vmax = mybir.AluOpType.max
    copyf = mybir.ActivationFunctionType.Copy

    with tile.TileContext(nc) as tc:
        with (
            tc.tile_pool(name="wpool", bufs=1) as wpool,
            tc.tile_pool(name="xpool", bufs=x_bufs) as xpool,
            tc.tile_pool(name="hpool", bufs=h_bufs) as hpool,
            tc.tile_pool(name="spool", bufs=s_bufs) as spool,
            tc.tile_pool(name="opool", bufs=o_bufs) as opool,
            tc.tile_pool(name="ps", bufs=ps_bufs, space="PSUM") as pspool,
        ):
            # Resident weights: per-(pair,kt) tiles so matmuls can start as
            # soon as their own 128KB slice lands.
            w1sb = wpool.tile([P, PAIRS, KT, 2 * H], BF16, tag="w1")
            for kt in range(KT):
                for pr in range(PAIRS):
                    nc.sync.dma_start(w1sb[:, pr, kt], w1t[pr, kt])
            nb1sb = wpool.tile([P, PAIRS, 2 * H], BF16, tag="nb1")
            nc.sync.dma_start(nb1sb[:], nb1[:])
            w2sb = wpool.tile([P, PAIRS, 2 * H], BF16, tag="w2")
            nc.sync.dma_start(w2sb[:], w2r[:])
            b2sb = wpool.tile([P, LPC], F32, tag="b2")
            nc.sync.dma_start(b2sb[:], b2p[:])

            vx = xT.rearrange("(k p) b -> p k b", p=P)

            def body(_=None):
                for bt in range(NBT):
                    bs = bt * P
                    xk = xpool.tile([P, KT, P], BF16, tag="xk")
                    nc.sync.dma_start(xk[:], vx[:, :, bs : bs + P])
                    ps = []
                    for pr in range(PAIRS):
                        ps.append(pspool.tile([P, 2 * H], F32, name="ps"))
                    for pr in range(PAIRS):
                        for kt in range(KT):
                            nc.tensor.matmul(
                                ps[pr][:],
                                xk[:, kt],
                                w1sb[:, pr, kt],
                                start=(kt == 0),
                                stop=(kt == KT - 1),
                            )
                    ot = opool.tile([P, LPC], F32)
                    for pr in range(PAIRS):
                        hm = hpool.tile([P, 2 * H], BF16, tag="hmax")
                        nc.vector.tensor_tensor(hm[:], ps[pr][:], nb1sb[:, pr], vmax)
                        prod = hpool.tile([P, 2 * H], BF16, tag="prod")
                        nc.vector.tensor_tensor(prod[:], hm[:], w2sb[:, pr], mult)
                        for l2 in range(2):
                            l = 2 * pr + l2
                            scr = spool.tile([P, H], BF16, tag="scr")
                            nc.scalar.activation(
                                scr[:],
                                prod[:, l2 * H : (l2 + 1) * H],
                                copyf,
                                accum_out=ot[:, l : l + 1],
                            )
                    ot2 = opool.tile([P, LPC], F32)
                    nc.vector.tensor_tensor(ot2[:], ot[:], b2sb[:], add)
                    nc.sync.dma_start(out[bs : bs + P, :], ot2[:])

            if repeat > 1:
                with tc.For_i(0, repeat) as _i:
                    body()
            else:
                body()

    nc.compile()
    return nc


def _build_a(repeat=1):
    """bf16 version of the previous W-stationary schedule (fallback)."""
    nc = bacc.Bacc(None, target_bir_lowering=False)

    xT = nc.dram_tensor("xT", [D, B], BF16, kind="ExternalInput")
    w1t = nc.dram_tensor("w1t", [LPC, HC, KT, P, P], BF16, kind="ExternalInput")
    w2blk = nc.dram_tensor("w2blk", [NL2, P, LPC], BF16, kind="ExternalInput")
    b1t = nc.dram_tensor("b1t", [NL2, P], F32, kind="ExternalInput")
    b2c = nc.dram_tensor("b2c", [LPC, 1], F32, kind="ExternalInput")
    out = nc.dram_tensor("out", [LPC, B], F32, kind="ExternalOutput")

    relu = mybir.ActivationFunctionType.Relu
    ident = mybir.ActivationFunctionType.Identity

    with tile.TileContext(nc) as tc:
        with (
            tc.tile_pool(name="wpool", bufs=1) as wpool,
            tc.tile_pool(name="xpool", bufs=16) as xpool,
            tc.tile_pool(name="hpool", bufs=18) as hpool,
            tc.tile_pool(name="opool", bufs=4) as opool,
            tc.tile_pool(name="ps1", bufs=5, space="PSUM") as ps1pool,
            tc.tile_pool(name="ps2", bufs=2, space="PSUM") as ps2pool,
        ):
            w1sb = []
            for l in range(LPC):
                t = wpool.tile([P, HC, KT, P], BF16, tag=f"w1_{l}")
                for hc in range(HC):
                    nc.sync.dma_start(t[:, hc], w1t[l, hc].rearrange("k i j -> i k j"))
                w1sb.append(t)
            w2sb = wpool.tile([P, NL2, LPC], BF16, tag="w2")
            nc.sync.dma_start(w2sb[:], w2blk.rearrange("n p j -> p n j"))
            b1sb = wpool.tile([P, NL2], F32, tag="b1")
            nc.sync.dma_start(b1sb[:], b1t.rearrange("n p -> p n"))
            b2sb = wpool.tile([LPC, 1], F32, tag="b2")
            nc.sync.dma_start(b2sb[:], b2c[:])

            xT_t = xT.rearrange("(k p) b -> p k b", p=P)

            def body(_=None):
                for c in range(NB):
                    bs = c * BCHUNK
                    xk = []
                    for kt in range(KT):
                        t = xpool.tile([P, BCHUNK], BF16, tag="xk")
                        nc.sync.dma_start(t[:], xT_t[:, kt, bs : bs + BCHUNK])
                        xk.append(t)
                    ps2 = ps2pool.tile([LPC, BCHUNK], F32)
                    hts = []
                    for l in range(LPC):
                        for hc in range(HC):
                            idx = l * HC + hc
                            ps1 = ps1pool.tile([P, BCHUNK], F32)
                            for kt in range(KT):
                                nc.tensor.matmul(
                                    ps1[:],
                                    w1sb[l][:, hc, kt],
                                    xk[kt][:],
                                    start=(kt == 0),
                                    stop=(kt == KT - 1),
                                )
                            ht = hpool.tile([P, BCHUNK], BF16)
                            nc.scalar.activation(
                                ht[:], ps1[:], relu, bias=b1sb[:, idx : idx + 1]
                            )
                            hts.append(ht)
                    for idx, ht in enumerate(hts):
                        nc.tensor.matmul(
                            ps2[:],
                            w2sb[:, idx],
                            ht[:],
                            start=(idx == 0),
                            stop=(idx == NL2 - 1),
                        )
                    ot = opool.tile([LPC, BCHUNK], F32)
                    nc.scalar.activation(ot[:], ps2[:], ident, bias=b2sb[:, 0:1])
                    nc.sync.dma_start(out[:, bs : bs + BCHUNK], ot[:])

            if repeat > 1:
                with tc.For_i(0, repeat) as _i:
                    body()
            else:
                body()

    nc.compile()
    return nc


def make_in_maps(x, W1, b1, W2, b2, variant="b"):
    """Shard + lay out the full fp32 inputs into per-core input maps."""
    x = np.asarray(x, dtype=np.float32)
    W1 = np.asarray(W1, dtype=np.float32)
    b1 = np.asarray(b1, dtype=np.float32)
    W2 = np.asarray(W2, dtype=np.float32)
    b2 = np.asarray(b2, dtype=np.float32)

    xT = np.ascontiguousarray(x.T).astype(NP_BF16)  # [D, B], shared replica
    in_maps = []
    for core in range(NCORES):
        sl = slice(core * LPC, (core + 1) * LPC)
        w1s = W1[sl]  # [LPC, D, H]
        b1s = b1[sl]  # [LPC, H]
        w2s = W2[sl]  # [LPC, H]
        b2s = b2[sl]  # [LPC]
        if variant == "a":
            w1tile = np.ascontiguousarray(
                w1s.reshape(LPC, KT, P, HC, P).transpose(0, 3, 1, 2, 4)
            ).astype(NP_BF16)
            w2b = np.zeros((NL2, P, LPC), dtype=NP_BF16)
            for l in range(LPC):
                for hc in range(HC):
                    w2b[l * HC + hc, :, l] = w2s[l, hc * P : (hc + 1) * P]
            in_maps.append(
                {
                    "xT": xT,
                    "w1t": w1tile,
                    "w2blk": w2b,
                    "b1t": np.ascontiguousarray(b1s.reshape(NL2, P)),
                    "b2c": np.ascontiguousarray(b2s.reshape(LPC, 1)),
                }
            )
            continue
        # variant b: [LPC, D, H] -> [PAIRS, KT, 128(d), 512(2l x 256h)]
        w1tile = np.ascontiguousarray(
            w1s.reshape(PAIRS, 2, KT, P, H).transpose(0, 2, 3, 1, 4)
            .reshape(PAIRS, KT, P, 2 * H)
        ).astype(NP_BF16)
        # -b1 in bf16, replicated across partitions; fold sum_h w2*b1(bf16)
        # into b2 so max(p,-b1)+b1 == relu(p+b1) exactly cancels.
        b1q = b1s.astype(NP_BF16)  # [LPC, H]
        nb1c = np.ascontiguousarray(
            np.broadcast_to(
                (-b1q).reshape(1, PAIRS, 2 * H), (P, PAIRS, 2 * H)
            )
        )
        w2q = w2s.astype(NP_BF16)
        w2rep = np.ascontiguousarray(
            np.broadcast_to(w2q.reshape(1, PAIRS, 2 * H), (P, PAIRS, 2 * H))
        )
        corr = (w2q.astype(np.float32) * b1q.astype(np.float32)).sum(axis=1)
        b2prime = (b2s + corr).astype(np.float32)  # [LPC]
        b2rep = np.ascontiguousarray(np.broadcast_to(b2prime.reshape(1, LPC), (P, LPC)))
        in_maps.append(
            {
                "xT": xT,
                "w1t": w1tile,
                "nb1": nb1c,
                "w2r": w2rep,
                "b2p": b2rep,
            }
        )
    return in_maps


def kernel(x, W1, b1, W2, b2):
    nc = build_nc()
    in_maps = make_in_maps(x, W1, b1, W2, b2)
    res = run_bass_kernel_spmd(nc, in_maps, core_ids=list(range(NCORES)))
    outs = [res.results[c]["out"] for c in range(NCORES)]  # each [B, LPC]
    return np.ascontiguousarray(np.concatenate(outs, axis=1)).astype(np.float32)


# revision 11
# speedup vs baseline: 1.2273x; 1.2273x over previous
"""Label-wise FFN kernel for Trainium2 (8 NeuronCores, label-sharded).

Computes out[b, l] = relu(x @ W1[l] + b1[l]) @ W2[l] + b2[l] for
B=8192, D=1024, L=64, H=256, fp32 in/out. rel-tol 2e-2 allows bf16
matmul operands (measured ~3e-3 end-to-end).

Sharding: L is split across the 8 cores (8 labels each); every core holds a
full replica of x. Per-core output is [B, 8]; host concatenates along L.

Per-core dataflow (variant "b", x-stationary):
  layer 1: for each b-tile (128 rows), stationary xT-tile [d=128, b=128]
           streams W1 moving tiles [d=128, 512] (2 labels x 256h packed on
           the free dim) -> psum[b=128, 512] accumulated over 8 d-tiles.
           bf16 operands enable fast-weight-load so the PE stays at the
           ~213ns/512-col streaming rate (f32r paid ~280ns fused loads).
  epilog:  relu(p + b1) = max(p, -b1) + b1, and sum_h w2*b1 is folded into
           b2 on the host. So DVE does: hmax = max(psum, -b1) (PSUM->SBUF,
           bf16), then per label a fused tensor_tensor_reduce
           out[b,l] = sum_h hmax*w2 + b2'. Layer 2 never touches the PE,
           which only does the 2048 layer-1 matmuls (~437us streaming
           floor).

Variant "a" is the previous schedule (W-stationary, h on partitions,
layer 2 as block-diagonal PE matmuls) with operands in bf16.
"""

import numpy as np
import ml_dtypes

import concourse.bacc as bacc
import concourse.mybir as mybir
import concourse.tile as tile
from concourse.bass_utils import run_bass_kernel_spmd

B, D, L, H = 8192, 1024, 64, 256
NCORES = 8
LPC = L // NCORES      # labels per core
P = 128
KT = D // P            # k-tiles over D
PAIRS = LPC // 2       # 2 labels (2x256h) packed per 512-wide moving tile
NBT = B // P           # b-tiles (variant b)
BCHUNK = 512
NB = B // BCHUNK       # b-chunks (variant a)
HC = H // P            # h-chunks per label (variant a)
NL2 = LPC * HC

BF16 = mybir.dt.bfloat16
F32 = mybir.dt.float32
NP_BF16 = ml_dtypes.bfloat16


def build_nc(variant="b", repeat=1, x_bufs=3, ps_bufs=8, h_bufs=6, s_bufs=3,
             o_bufs=3):
    if variant == "a":
        return _build_a(repeat)
    nc = bacc.Bacc(None, target_bir_lowering=False)

    xT = nc.dram_tensor("xT", [D, B], BF16, kind="ExternalInput")
    w1t = nc.dram_tensor("w1t", [PAIRS, KT, P, 2 * H], BF16, kind="ExternalInput")
    nb1 = nc.dram_tensor("nb1", [P, PAIRS, 2 * H], BF16, kind="ExternalInput")
    w2r = nc.dram_tensor("w2r", [P, PAIRS, 2 * H], BF16, kind="ExternalInput")
    b2p = nc.dram_tensor("b2p", [P, LPC], F32, kind="ExternalInput")
    out = nc.dram_tensor("out", [B, LPC], F32, kind="ExternalOutput")

    mult = mybir.AluOpType.mult
    add = mybir.AluOpType.add
    vmax = mybir.AluOpType.max
    copyf = mybir.ActivationFunctionType.Copy

    with tile.TileContext(nc) as tc:
        with (
            tc.tile_pool(name="wpool", bufs=1) as wpool,
            tc.tile_pool(name="xpool", bufs=x_bufs) as xpool,
            tc.tile_pool(name="hpool", bufs=h_bufs) as hpool,
            tc.tile_pool(name="spool", bufs=s_bufs) as spool,
            tc.tile_pool(name="opool", bufs=o_bufs) as opool,
            tc.tile_pool(name="ps", bufs=ps_bufs, space="PSUM") as pspool,
        ):
            # Resident weights: per-(pair,kt) tiles so matmuls can start as
            # soon as their own 128KB slice lands.
            w1sb = wpool.tile([P, PAIRS, KT, 2 * H], BF16, tag="w1")
            for kt in range(KT):
                for pr in range(PAIRS):
                    nc.sync.dma_start(w1sb[:, pr, kt], w1t[pr, kt])
            nb1sb = wpool.tile([P, PAIRS, 2 * H], BF16, tag="nb1")
            nc.sync.dma_start(nb1sb[:], nb1[:])
            w2sb = wpool.tile([P, PAIRS, 2 * H], BF16, tag="w2")
            nc.sync.dma_start(w2sb[:], w2r[:])
            b2sb = wpool.tile([P, LPC], F32, tag="b2")
            nc.sync.dma_start(b2sb[:], b2p[:])

            # x fetched in [128d, 8kt, 1024b] slabs: 2KB contiguous lines per
            # partition (a 128-wide b-tile fetch would be 256B lines, which
            # starves the PE — measured DMA-bound at ~600us).
            BSLAB = 1024
            NBC = B // BSLAB
            BT_PER = BSLAB // P
            vx = xT.rearrange("(k p) (c b) -> p k c b", p=P, b=BSLAB)

            def body(_=None):
                for bc in range(NBC):
                    xk = xpool.tile([P, KT, BSLAB], BF16, tag="xk")
                    nc.sync.dma_start(xk[:], vx[:, :, bc, :])
                    for bt8 in range(BT_PER):
                        bs = (bc * BT_PER + bt8) * P
                        ps = []
                        for pr in range(PAIRS):
                            ps.append(pspool.tile([P, 2 * H], F32, name="ps"))
                        for pr in range(PAIRS):
                            for kt in range(KT):
                                nc.tensor.matmul(
                                    ps[pr][:],
                                    xk[:, kt, bt8 * P : (bt8 + 1) * P],
                                    w1sb[:, pr, kt],
                                    start=(kt == 0),
                                    stop=(kt == KT - 1),
                                )
                        ot = opool.tile([P, LPC], F32)
                        for pr in range(PAIRS):
                            hm = hpool.tile([P, 2 * H], BF16, tag="hmax")
                            nc.vector.tensor_tensor(
                                hm[:], ps[pr][:], nb1sb[:, pr], vmax
                            )
                            prod = hpool.tile([P, 2 * H], BF16, tag="prod")
                            nc.vector.tensor_tensor(prod[:], hm[:], w2sb[:, pr], mult)
                            for l2 in range(2):
                                l = 2 * pr + l2
                                scr = spool.tile([P, H], BF16, tag="scr")
                                nc.scalar.activation(
                                    scr[:],
                                    prod[:, l2 * H : (l2 + 1) * H],
                                    copyf,
                                    accum_out=ot[:, l : l + 1],
                                )
                        ot2 = opool.tile([P, LPC], F32)
                        nc.vector.tensor_tensor(ot2[:], ot[:], b2sb[:], add)
                        nc.sync.dma_start(out[bs : bs + P, :], ot2[:])

            if repeat > 1:
                with tc.For_i(0, repeat) as _i:
                    body()
            else:
                body()

    nc.compile()
    return nc


def _build_a(repeat=1):
    """bf16 version of the previous W-stationary schedule (fallback)."""
    nc = bacc.Bacc(None, target_bir_lowering=False)

    xT = nc.dram_tensor("xT", [D, B], BF16, kind="ExternalInput")
    w1t = nc.dram_tensor("w1t", [LPC, HC, KT, P, P], BF16, kind="ExternalInput")
    w2blk = nc.dram_tensor("w2blk", [NL2, P, LPC], BF16, kind="ExternalInput")
    b1t = nc.dram_tensor("b1t", [NL2, P], F32, kind="ExternalInput")
    b2c = nc.dram_tensor("b2c", [LPC, 1], F32, kind="ExternalInput")
    out = nc.dram_tensor("out", [LPC, B], F32, kind="ExternalOutput")

    relu = mybir.ActivationFunctionType.Relu
    ident = mybir.ActivationFunctionType.Identity

    with tile.TileContext(nc) as tc:
        with (
            tc.tile_pool(name="wpool", bufs=1) as wpool,
            tc.tile_pool(name="xpool", bufs=16) as xpool,
            tc.tile_pool(name="hpool", bufs=18) as hpool,
            tc.tile_pool(name="opool", bufs=4) as opool,
            tc.tile_pool(name="ps1", bufs=5, space="PSUM") as ps1pool,
            tc.tile_pool(name="ps2", bufs=2, space="PSUM") as ps2pool,
        ):
            w1sb = []
            for l in range(LPC):
                t = wpool.tile([P, HC, KT, P], BF16, tag=f"w1_{l}")
                for hc in range(HC):
                    nc.sync.dma_start(t[:, hc], w1t[l, hc].rearrange("k i j -> i k j"))
                w1sb.append(t)
            w2sb = wpool.tile([P, NL2, LPC], BF16, tag="w2")
            nc.sync.dma_start(w2sb[:], w2blk.rearrange("n p j -> p n j"))
            b1sb = wpool.tile([P, NL2], F32, tag="b1")
            nc.sync.dma_start(b1sb[:], b1t.rearrange("n p -> p n"))
            b2sb = wpool.tile([LPC, 1], F32, tag="b2")
            nc.sync.dma_start(b2sb[:], b2c[:])

            xT_t = xT.rearrange("(k p) b -> p k b", p=P)

            def body(_=None):
                for c in range(NB):
                    bs = c * BCHUNK
                    xk = []
                    for kt in range(KT):
                        t = xpool.tile([P, BCHUNK], BF16, tag="xk")
                        nc.sync.dma_start(t[:], xT_t[:, kt, bs : bs + BCHUNK])
                        xk.append(t)
                    ps2 = ps2pool.tile([LPC, BCHUNK], F32)
                    hts = []
                    for l in range(LPC):
                        for hc in range(HC):
                            idx = l * HC + hc
                            ps1 = ps1pool.tile([P, BCHUNK], F32)
                            for kt in range(KT):
                                nc.tensor.matmul(
                                    ps1[:],
                                    w1sb[l][:, hc, kt],
                                    xk[kt][:],
                                    start=(kt == 0),
                                    stop=(kt == KT - 1),
                                )
                            ht = hpool.tile([P, BCHUNK], BF16)
                            nc.scalar.activation(
                                ht[:], ps1[:], relu, bias=b1sb[:, idx : idx + 1]
                            )
                            hts.append(ht)
                    for idx, ht in enumerate(hts):
                        nc.tensor.matmul(
                            ps2[:],
                            w2sb[:, idx],
                            ht[:],
                            start=(idx == 0),
                            stop=(idx == NL2 - 1),
                        )
                    ot = opool.tile([LPC, BCHUNK], F32)
                    nc.scalar.activation(ot[:], ps2[:], ident, bias=b2sb[:, 0:1])
                    nc.sync.dma_start(out[:, bs : bs + BCHUNK], ot[:])

            if repeat > 1:
                with tc.For_i(0, repeat) as _i:
                    body()
            else:
                body()

    nc.compile()
    return nc


def make_in_maps(x, W1, b1, W2, b2, variant="b"):
    """Shard + lay out the full fp32 inputs into per-core input maps."""
    x = np.asarray(x, dtype=np.float32)
    W1 = np.asarray(W1, dtype=np.float32)
    b1 = np.asarray(b1, dtype=np.float32)
    W2 = np.asarray(W2, dtype=np.float32)
    b2 = np.asarray(b2, dtype=np.float32)

    xT = np.ascontiguousarray(x.T).astype(NP_BF16)  # [D, B], shared replica
    in_maps = []
    for core in range(NCORES):
        sl = slice(core * LPC, (core + 1) * LPC)
        w1s = W1[sl]  # [LPC, D, H]
        b1s = b1[sl]  # [LPC, H]
        w2s = W2[sl]  # [LPC, H]
        b2s = b2[sl]  # [LPC]
        if variant == "a":
            w1tile = np.ascontiguousarray(
                w1s.reshape(LPC, KT, P, HC, P).transpose(0, 3, 1, 2, 4)
            ).astype(NP_BF16)
            w2b = np.zeros((NL2, P, LPC), dtype=NP_BF16)
            for l in range(LPC):
                for hc in range(HC):
                    w2b[l * HC + hc, :, l] = w2s[l, hc * P : (hc + 1) * P]
            in_maps.append(
                {
                    "xT": xT,
                    "w1t": w1tile,
                    "w2blk": w2b,
                    "b1t": np.ascontiguousarray(b1s.reshape(NL2, P)),
                    "b2c": np.ascontiguousarray(b2s.reshape(LPC, 1)),
                }
            )
            continue
        # variant b: [LPC, D, H] -> [PAIRS, KT, 128(d), 512(2l x 256h)]
        w1tile = np.ascontiguousarray(
            w1s.reshape(PAIRS, 2, KT, P, H).transpose(0, 2, 3, 1, 4)
            .reshape(PAIRS, KT, P, 2 * H)
        ).astype(NP_BF16)
        # -b1 in bf16, replicated across partitions; fold sum_h w2*b1(bf16)
        # into b2 so max(p,-b1)+b1 == relu(p+b1) exactly cancels.
        b1q = b1s.astype(NP_BF16)  # [LPC, H]
        nb1c = np.ascontiguousarray(
            np.broadcast_to(
                (-b1q).reshape(1, PAIRS, 2 * H), (P, PAIRS, 2 * H)
            )
        )
        w2q = w2s.astype(NP_BF16)
        w2rep = np.ascontiguousarray(
            np.broadcast_to(w2q.reshape(1, PAIRS, 2 * H), (P, PAIRS, 2 * H))
        )
        corr = (w2q.astype(np.float32) * b1q.astype(np.float32)).sum(axis=1)
        b2prime = (b2s + corr).astype(np.float32)  # [LPC]
        b2rep = np.ascontiguousarray(np.broadcast_to(b2prime.reshape(1, LPC), (P, LPC)))
        in_maps.append(
            {
                "xT": xT,
                "w1t": w1tile,
                "nb1": nb1c,
                "w2r": w2rep,
                "b2p": b2rep,
            }
        )
    return in_maps


def kernel(x, W1, b1, W2, b2):
    nc = build_nc()
    in_maps = make_in_maps(x, W1, b1, W2, b2)
    res = run_bass_kernel_spmd(nc, in_maps, core_ids=list(range(NCORES)))
    outs = [res.results[c]["out"] for c in range(NCORES)]  # each [B, LPC]
    return np.ascontiguousarray(np.concatenate(outs, axis=1)).astype(np.float32)
